# revision 28
# baseline (speedup 1.0000x reference)
"""Trainium2 Bass kernel for nn_RandProjector (histogram_binning).

Computes, for x [16384, 1024] and W [6400, 1024]:
    proj = x @ W.T                      # [S, D] -- never materialized in HBM
    per-column 20-bin histogram of proj (torch.histc semantics with
    mins/maxs as ranges), reshaped [100, 64, 20], L2-normalized over bins.

Strategy (8 NeuronCores, data-parallel over S):
  - Host pre-transposes x and W (plain multi-queue DMA loads, no xbar
    transpose serialization) and scales W rows by bins/width_d so the
    fp16 matmul accumulates proj' = proj * bins/width in PSUM fp32.
  - ScalarE stages each PSUM tile to SBUF fp16 with a per-partition bias
    (Identity activation): u = proj' - min*bins/width in [0, 20].  This
    frees the PSUM slot after ~2us so the PE never stalls on compare
    progress, and makes all 19 bin edges integer constants.
  - cdf_b = #(u >= b) via fused compare+accumulate.  The DVE accumulate
    path runs at 1x regardless of dtype (HW-measured), so VectorE uses a
    hand-built triple-edge DVE op (registered below): the accumulator
    carries c(b) + 4096*c(b+5) (two exact counts packed into one fp32),
    while a third compare against b+10 (computed by the seed uop into a
    swap flop) keeps a running count in a spare pipeline stage and
    streams it to the out tensor -- its last element is the third count,
    harvested for all five passes with one strided copy per tile.
    15 edges in 5 VectorE passes; ScalarE handles edges 16..19 via Sign
    activation (+accum) plus the PSUM->SBUF staging.
  - Bin counts are cdf differences; bin 0 uses the constant shard total.
  - Histogram pieces AllReduce across the 8 cores as soon as their tiles
    finish (overlapping compute); the last piece is small so the
    trailing collective is short.  L2-normalize on device.
"""

import sys

if "/opt/trn_rl_repo" not in sys.path:
    sys.path.insert(0, "/opt/trn_rl_repo")

import numpy as np

S, IN_DIM = 16384, 1024
NUM_PROJ, PROJ_DIM, BINS = 100, 64, 20
D = NUM_PROJ * PROJ_DIM          # 6400
N_CORES = 8
S_SHARD = S // N_CORES           # 2048
NE = BINS - 1                    # 19 interior edges (b = 1..19)
NP3 = 5                          # triple-edge passes on VectorE
NA = 4                           # Sign edges 16..19 on ScalarE
PK = 4096.0                      # dual-edge packing factor

_CACHE = {}


def _cdf3_ref(in0, in1, s0, s1, imm2):
    x = in0.astype(np.float32)
    e3 = 2.0 * s1 - s0
    dual = (x >= s0).astype(np.float32) + (x >= s1).astype(np.float32) * imm2
    pre = np.cumsum((x >= e3).astype(np.float32).reshape(x.shape[0], -1),
                    axis=-1).reshape(x.shape)
    return pre, dual.reshape(x.shape[0], -1).sum(axis=-1, keepdims=True)


def _register_cdf3():
    """Triple-edge compare op: accum = c(s0) + 4096*c(s1) (packed dual),
    out stream = running count of (x >= 2*s1-s0) -- its last element is the
    third edge's count.  Built by extending the lowered dual program's free
    BYPASS stages (idempotent registration)."""
    import copy as _copy
    from operator import add

    from concourse import dve_ops
    from concourse.dve_spec import C0, C1, C2, Spec, Src0, Zero, lower
    from concourse.dve_uop import (AluInp, AluOp, DelayInp, DveOpSpec,
                                   InpSel, OutSel)

    if "CDF3_ANT" in dve_ops._SUB_OPCODE_FOR_NAME:
        return next(op for op in dve_ops.OPS if op.name == "CDF3_ANT")
    spec = Spec(
        body=(Src0 >= C0) + (Src0 >= C1) * C2,
        accum=add,
        accum_init=Zero,
        reference=_cdf3_ref,
    )
    seed, steady = (_copy.deepcopy(u) for u in lower(spec, ver="v3"))

    PA, CA = AluInp.PREV_ALU_OUT, AluInp.CURR_ALU_OUT
    PD = [AluInp.PREV_DELAY_0, AluInp.PREV_DELAY_1, AluInp.PREV_DELAY_2,
          AluInp.PREV_DELAY_3, AluInp.PREV_DELAY_4]

    # seed: compute e3 = C1+C1-C0, carry it on delay lane 0 (s2 captures
    # s1's ALU out there already), latch into s5's swap flop; keep s4's
    # zero-seeding of the dual accumulator; zero s6's scan flop via lane 4.
    seed.datapath_config[0].op = AluOp.ADD
    seed.datapath_config[0].alu_src0 = PD[2]      # C1
    seed.datapath_config[0].alu_src1 = PD[2]
    seed.datapath_config[1].op = AluOp.SUBTRACT
    seed.datapath_config[1].alu_src0 = PA         # 2*C1
    seed.datapath_config[1].alu_src1 = PD[1]      # C0
    seed.datapath_config[4].delay[0] = DelayInp.PREV_DELAY   # keep e3 on L0
    seed.datapath_config[5].op = AluOp.BYPASS
    seed.datapath_config[5].alu_src0 = PD[0]      # e3
    seed.datapath_config[5].alu_src1 = PD[0]
    seed.datapath_config[5].swap_enable = 1       # latch e3
    seed.datapath_config[6].op = AluOp.BYPASS
    seed.datapath_config[6].alu_src0 = PD[4]      # lane4 = zero at seed
    seed.datapath_config[6].alu_src1 = PD[4]
    # s5 outputs e3 (don't write A there); s6/s7 output zero and keep the
    # original A-reset-through-the-tail behaviour
    seed.datapath_config[5].alu_out_a_enable = 0

    # steady: lane 5 carries x (PD4); s5 compares vs the latched e3,
    # s6 keeps the running count (scan), s7 forwards it to the output.
    steady.inp[5] = InpSel.SRC_0
    steady.datapath_config[5].op = AluOp.IS_GE
    steady.datapath_config[5].alu_src0 = PD[4]    # x
    steady.datapath_config[5].alu_src1 = AluInp.CURR_SWAP_OUT
    steady.datapath_config[5].alu_out_a_enable = 0
    # the dual running sum (s4's ALU out) rides delay lane 0 to s7 so the
    # accumulator readback still sees it carried through the pipeline tail
    steady.datapath_config[5].delay[0] = DelayInp.PREV_ALU_OUT
    steady.datapath_config[6].op = AluOp.ADD
    steady.datapath_config[6].alu_src0 = CA       # running 3rd-edge count
    steady.datapath_config[6].alu_src1 = PA
    steady.datapath_config[6].alu_out_a_enable = 0
    steady.datapath_config[6].delay[0] = DelayInp.PREV_DELAY
    steady.datapath_config[7].op = AluOp.BYPASS
    steady.datapath_config[7].alu_src0 = PD[0]    # dual running sum
    steady.datapath_config[7].alu_src1 = PD[0]
    steady.datapath_config[7].alu_out_a_enable = 1
    steady.datapath_config[7].delay[1] = DelayInp.PREV_ALU_OUT  # scan value
    steady.out_enable = {k: (1 if k.name == "WR0_LO" else 0)
                         for k in steady.out_enable}
    steady.out = {k: OutSel.DELAY_1 for k in steady.out}

    for u in (seed, steady):
        u.validate("v3")
    row = max(dve_ops._SUB_OPCODE_FOR_NAME.values()) + 1
    assert row < 0x20
    compiled = DveOpSpec(name="CDF3_ANT", opcode=row, uops=[seed, steady],
                         rd1_en=False)

    class _TripleDveOp(dve_ops.DveOp):
        def compile(self, ver):
            assert ver == "v3", f"hand-built uops are v3-only, got {ver}"
            return compiled

    op = _TripleDveOp("CDF3_ANT", spec, subdim=False,
                      uops_sha={"v3": compiled.sha("v3")})
    dve_ops.OPS.append(op)
    dve_ops.CUSTOM_DVE_SPECS[op.name] = op.spec
    dve_ops._SUB_OPCODE_FOR_NAME[op.name] = row
    return op


def build(s_shard=S_SHARD, d=D, in_dim=IN_DIM, n_cores=N_CORES, debug=False):
    import concourse.bacc as bacc
    import concourse.bass as bass
    from concourse import mybir
    from concourse.tile import TileContext

    cdf3 = _register_cdf3()

    f32 = mybir.dt.float32
    f16 = mybir.dt.float16
    i32 = mybir.dt.int32
    nt = d // 128                # 50
    kc_n = in_dim // 128         # 8
    chw = min(512, s_shard)      # matmul moving-operand width
    nch_n = s_shard // chw       # 4

    nc = bacc.Bacc("TRN2", target_bir_lowering=False, debug=False,
                   num_devices=n_cores)

    xT_d = nc.dram_tensor("xT16", [kc_n, 128, s_shard], f16,
                          kind="ExternalInput")
    wT_d = nc.dram_tensor("wT16", [kc_n, 128, d], f16, kind="ExternalInput")
    bias_d = nc.dram_tensor("bias", [128, nt], f32, kind="ExternalInput")
    out_d = nc.dram_tensor("out", [d, BINS], f32, kind="ExternalOutput")
    # pieces of the histogram all-reduce independently, overlapping compute;
    # the last piece is small so the trailing collective is short
    pieces = [(0, 22), (22, 40), (40, 47), (47, nt)]
    cc_ins, cc_outs = [], []
    for i, (t0, t1) in enumerate(pieces):
        cc_ins.append(nc.dram_tensor(f"cc_in{i}", [128, (t1 - t0) * BINS], f32))
        cc_outs.append(nc.dram_tensor(f"cc_out{i}", [128, (t1 - t0) * BINS],
                                      f32, addr_space="Shared"))
    if debug:
        dbg_hist = nc.dram_tensor("dbg_hist", [128, nt, BINS], f32,
                                  kind="ExternalOutput")
        dbg_cdf = nc.dram_tensor("dbg_cdf", [128, nt, BINS + 1], f32,
                                 kind="ExternalOutput")

    with TileContext(nc) as tc:
        with (
            tc.tile_pool(name="singles", bufs=1) as singles,
            tc.tile_pool(name="sp_pool", bufs=3) as sp_pool,
            tc.tile_pool(name="ps_p", bufs=2, space="PSUM") as ps_p,
        ):
            bias_sb = singles.tile([128, nt], f32)
            nc.sync.dma_start(out=bias_sb, in_=bias_d[:, :])

            # ScalarE Sign biases (-b for b = 16..19), one column each
            half_n = singles.tile([128, 1], f32)
            nc.vector.memset(half_n, float(s_shard) / 2)
            negb = singles.tile([128, NA], f32)
            for j in range(NA):
                nc.vector.memset(negb[:, j:j + 1], float(-(16 + j)))

            # per-pass scan outputs (counts <= 2048 are fp16-exact);
            # one strided copy harvests all five last-elements per tile
            vout = singles.tile([128, NP3, s_shard], f16)
            trash_a = singles.tile([128, s_shard], f16)

            # per-engine cdf accumulators (separate tiles so Tile never
            # serializes VectorE against ScalarE on writes)
            accp = singles.tile([128, nt, NP3], f32)   # packed dual counts
            acc3 = singles.tile([128, nt, NP3], f32)   # harvested 3rd counts
            acc_a = singles.tile([128, nt, NA], f32)

            # ---- Phase 0: plain DMA of pre-transposed x shard and W ----
            xT = singles.tile([128, kc_n, s_shard], f16)
            wT = singles.tile([128, kc_n, d], f16)
            # 16 fat x transfers (2 KB per partition line) spread the load
            # across all DMA queues; tile 0 only needs x + the first W chunk
            for kc in range(kc_n):
                for sh in range(2):
                    sl = slice(sh * (s_shard // 2), (sh + 1) * (s_shard // 2))
                    nc.sync.dma_start(out=xT[:, kc, sl], in_=xT_d[kc, :, sl])
            # W streams lazily in per-tile chunks behind the x load: tile 0
            # is gated only on x (4.2 MB) + the first 128 W columns, and
            # later chunks always arrive well ahead of their tile.
            for d0 in range(0, d, 128):
                for kc in range(kc_n):
                    nc.sync.dma_start(out=wT[:, kc, d0:d0 + 128],
                                      in_=wT_d[kc, :, d0:d0 + 128])

            # normalization / unpack scratch
            cdfx = singles.tile([128, nt, BINS + 1], f32)
            nc.vector.memset(cdfx[:, :, 0:1], float(s_shard))
            nc.vector.memset(cdfx[:, :, BINS:BINS + 1], 0.0)
            ip_t = singles.tile([128, nt, NP3], i32)
            ip2_t = singles.tile([128, nt, NP3], i32)
            hi_t = singles.tile([128, nt, NP3], f32)
            hist = singles.tile([128, nt, BINS], f32)
            hsum = singles.tile([128, nt, BINS], f32)
            sq = singles.tile([128, nt, BINS], f32)
            n2 = singles.tile([128, nt], f32)
            y_t = singles.tile([128, nt], f32)
            iy = singles.tile([128, nt], f32)
            a_t = singles.tile([128, nt], f32)
            b_t = singles.tile([128, nt], f32)
            r_t = singles.tile([128, nt], f32)
            outn = singles.tile([128, nt, BINS], f32)
            out_v = out_d[:, :].rearrange("(t p) b -> p t b", p=128)

            def emit_cc(hi):
                """Unpack cdf partials for tau in [t0, t1) and kick off the
                cross-core all-reduce (runs on DMA/CC queues in background)."""
                t0, t1 = pieces[hi]
                sl = slice(t0, t1)
                # unpack: lo = accp & 0xFFF (exact: accp is integer-valued
                # fp32 < 2^23, i32 convert exact), mid = (accp - lo) / 4096;
                # third-edge counts arrive pre-unpacked via the scan harvest
                nc.vector.tensor_copy(ip_t[:, sl], accp[:, sl])
                nc.vector.tensor_scalar(ip2_t[:, sl], ip_t[:, sl], 0xFFF,
                                        None, op0=mybir.AluOpType.bitwise_and)
                nc.vector.tensor_copy(cdfx[:, sl, 1:1 + NP3], ip2_t[:, sl])
                nc.vector.tensor_tensor(out=hi_t[:, sl], in0=accp[:, sl],
                                        in1=cdfx[:, sl, 1:1 + NP3],
                                        op=mybir.AluOpType.subtract)
                nc.vector.tensor_scalar(cdfx[:, sl, 1 + NP3:1 + 2 * NP3],
                                        hi_t[:, sl], 1.0 / PK, None,
                                        op0=mybir.AluOpType.mult)
                nc.vector.tensor_copy(cdfx[:, sl, 1 + 2 * NP3:1 + 3 * NP3],
                                      acc3[:, sl])
                # ScalarE counts are sums of sign in {-1,0,1}:
                # cdf = 0.5*sgn + N/2 -- computed on ScalarE, which has
                # more slack than VectorE here
                nc.scalar.activation(
                    out=cdfx[:, sl, 1 + 3 * NP3:BINS], in_=acc_a[:, sl],
                    func=mybir.ActivationFunctionType.Identity,
                    bias=half_n[:, 0:1], scale=0.5)
                nc.vector.tensor_tensor(
                    out=hist[:, sl], in0=cdfx[:, sl, 0:BINS],
                    in1=cdfx[:, sl, 1:BINS + 1],
                    op=mybir.AluOpType.subtract)
                if debug and hi == len(pieces) - 1:
                    nc.sync.dma_start(out=dbg_hist[:, :, :], in_=hist)
                    nc.sync.dma_start(out=dbg_cdf[:, :, :], in_=cdfx)
                nc.sync.dma_start(
                    out=cc_ins[hi][:, :],
                    in_=hist[:, sl].rearrange("p a b -> p (a b)"))
                nc.gpsimd.collective_compute(
                    "AllReduce",
                    mybir.AluOpType.add,
                    replica_groups=[list(range(n_cores))],
                    ins=[cc_ins[hi][:, :]],
                    outs=[cc_outs[hi][:, :]],
                )
                nc.sync.dma_start(
                    out=hsum[:, sl].rearrange("p a b -> p (a b)"),
                    in_=cc_outs[hi][:, :])

            hsum_g = singles.tile([128, nt, BINS], f32)

            def emit_norm(t0, t1, anchor=None):
                """L2-normalize the summed histogram for tau in [t0, t1) and
                write the output slice. When `anchor` is a tile index, route
                hsum through a no-op add of that tile's accumulator data so
                the scheduler's cost model places the chain after that tile's
                compares -- the collective is then long finished and no
                engine FIFO stalls on it."""
                sl = slice(t0, t1)
                w = t1 - t0
                if anchor is not None:
                    g_ap = acc_a[:, anchor, NA - 1:NA]
                    g_b = bass.AP(tensor=g_ap.tensor, offset=g_ap.offset,
                                  ap=[g_ap.ap[0], [0, w], [0, BINS]])
                    nc.vector.scalar_tensor_tensor(
                        out=hsum_g[:, sl], in0=g_b, scalar=0.0,
                        in1=hsum[:, sl],
                        op0=mybir.AluOpType.mult, op1=mybir.AluOpType.add)
                    h_in = hsum_g
                else:
                    h_in = hsum
                nc.vector.tensor_tensor(out=sq[:, sl], in0=h_in[:, sl],
                                        in1=h_in[:, sl],
                                        op=mybir.AluOpType.mult)
                nc.vector.tensor_reduce(out=n2[:, sl], in_=sq[:, sl],
                                        axis=mybir.AxisListType.X,
                                        op=mybir.AluOpType.add)
                nc.scalar.sqrt(y_t[:, sl], n2[:, sl])
                nc.vector.reciprocal(iy[:, sl], y_t[:, sl])
                # one Newton step for rsqrt: r = iy * (1.5 - 0.5*n2*iy^2)
                nc.vector.tensor_tensor(out=a_t[:, sl], in0=iy[:, sl],
                                        in1=iy[:, sl],
                                        op=mybir.AluOpType.mult)
                nc.vector.tensor_tensor(out=b_t[:, sl], in0=a_t[:, sl],
                                        in1=n2[:, sl],
                                        op=mybir.AluOpType.mult)
                nc.vector.tensor_scalar(b_t[:, sl], b_t[:, sl], -0.5, 1.5,
                                        op0=mybir.AluOpType.mult,
                                        op1=mybir.AluOpType.add)
                nc.vector.tensor_tensor(out=r_t[:, sl], in0=iy[:, sl],
                                        in1=b_t[:, sl],
                                        op=mybir.AluOpType.mult)
                r_ap = r_t[:, sl]
                r_b = bass.AP(tensor=r_ap.tensor, offset=r_ap.offset,
                              ap=[r_ap.ap[0], r_ap.ap[1], [0, BINS]])
                nc.vector.tensor_tensor(out=outn[:, sl], in0=h_in[:, sl],
                                        in1=r_b, op=mybir.AluOpType.mult)
                nc.sync.dma_start(out=out_v[:, sl], in_=outn[:, sl])

            # ---- Phase 1: d-tiles ----
            for tau in range(nt):
                pp = ps_p.tile([128, s_shard], f32)
                for nch in range(nch_n):
                    for kc in range(kc_n):
                        nc.tensor.matmul(
                            pp[:, nch * chw:(nch + 1) * chw],
                            lhsT=wT[:, kc, tau * 128:(tau + 1) * 128],
                            rhs=xT[:, kc, nch * chw:(nch + 1) * chw],
                            start=(kc == 0),
                            stop=(kc == kc_n - 1),
                        )
                # Stage PSUM -> SBUF fp16 once, adding the per-dim bias:
                # u = proj' - min * bins/width  in [0, bins].  Frees the
                # PSUM slot after ~2us so the PE is never gated on compares.
                sp = sp_pool.tile([128, s_shard], f16)
                nc.scalar.activation(
                    out=sp, in_=pp,
                    func=mybir.ActivationFunctionType.Identity,
                    bias=bias_sb[:, tau:tau + 1], scale=1.0)
                # VectorE: triple passes (b, b+5, b+10): packed dual in
                # the accumulator, third count harvested from the scan
                # output's last element before the next pass reuses trash_v
                for j in range(NP3):
                    nc.vector._custom_dve(
                        cdf3,
                        out=vout[:, j, :],
                        in0=sp,
                        s0=float(1 + j),
                        s1=float(6 + j),
                        imm2=PK,
                        accum_out=accp[:, tau, j:j + 1],
                    )
                vlast = vout[:, 0:NP3, s_shard - 1:s_shard]
                vl = bass.AP(tensor=vlast.tensor, offset=vlast.offset,
                             ap=[vlast.ap[0], [s_shard, NP3]])
                a3 = acc3[:, tau:tau + 1, 0:NP3]
                a3f = bass.AP(tensor=a3.tensor, offset=a3.offset,
                              ap=[a3.ap[0], [1, NP3]])
                nc.vector.tensor_copy(a3f, vl)
                # ScalarE: edges b = 16..19 via Sign(u - b)
                for j in range(NA):
                    nc.scalar.activation(
                        out=trash_a,
                        in_=sp,
                        func=mybir.ActivationFunctionType.Sign,
                        bias=negb[:, j:j + 1],
                        scale=1.0,
                        accum_out=acc_a[:, tau, j:j + 1],
                    )
                for hi, (t0, t1) in enumerate(pieces):
                    if tau == t1 - 1:
                        emit_cc(hi)
            # normalization staged so the bulk runs during the last tiles'
            # compute (its collectives have long completed); only the last
            # piece's collective is actually waited on at the end
            emit_norm(0, 40, anchor=47)
            emit_norm(40, pieces[-2][1], anchor=nt - 1)
            emit_norm(pieces[-1][0], pieces[-1][1])

    nc.compile()
    return nc


def host_prep(x, W, mins, maxs, s_shard=S_SHARD, n_cores=N_CORES):
    d = W.shape[0]
    nt = d // 128
    kc_n = IN_DIM // 128
    mins64 = np.asarray(mins, dtype=np.float64)
    maxs64 = np.asarray(maxs, dtype=np.float64)
    scale = float(BINS) / (maxs64 - mins64)                        # [d]
    # W rows scaled so the matmul yields proj' = proj * bins/width
    Ws = np.asarray(W, dtype=np.float64) * scale[:, None]
    w16T = np.ascontiguousarray(Ws.T.astype(np.float16))           # [in, d]
    w16T = w16T.reshape(kc_n, 128, d)
    x16T = np.ascontiguousarray(np.asarray(x, dtype=np.float16).T)  # [in, S]
    x16T = x16T.reshape(kc_n, 128, S)
    bias = (-mins64 * scale).astype(np.float32)                    # [d]
    bias_l = np.ascontiguousarray(bias.reshape(nt, 128).T)         # [128, nt]
    in_maps = []
    for i in range(n_cores):
        in_maps.append({
            "xT16": np.ascontiguousarray(
                x16T[:, :, i * s_shard:(i + 1) * s_shard]),
            "wT16": w16T,
            "bias": bias_l,
        })
    return in_maps


def run(x, W, mins, maxs, trace=False, **trace_kw):
    """Returns (output [100, 64, 20] f32, BassKernelResults)."""
    from concourse.bass_utils import run_bass_kernel_spmd

    if "nc" not in _CACHE:
        _CACHE["nc"] = build()
    nc = _CACHE["nc"]
    in_maps = host_prep(x, W, mins, maxs)
    res = run_bass_kernel_spmd(nc, in_maps, core_ids=list(range(N_CORES)),
                               trace=trace, **trace_kw)
    out = res.results[0]["out"].reshape(NUM_PROJ, PROJ_DIM, BINS)
    return np.asarray(out, dtype=np.float32), res


def kernel(x, W, mins, maxs, num_of_projection=NUM_PROJ, bins=BINS):
    assert int(num_of_projection) == NUM_PROJ and int(bins) == BINS
    out, _ = run(x, W, mins, maxs, trace=False)
    return out


# revision 30
# speedup vs baseline: 1.0029x; 1.0029x over previous
"""Trainium2 Bass kernel for nn_RandProjector (histogram_binning).

Computes, for x [16384, 1024] and W [6400, 1024]:
    proj = x @ W.T                      # [S, D] -- never materialized in HBM
    per-column 20-bin histogram of proj (torch.histc semantics with
    mins/maxs as ranges), reshaped [100, 64, 20], L2-normalized over bins.

Strategy (8 NeuronCores, data-parallel over S):
  - Host pre-transposes x and W (plain multi-queue DMA loads, no xbar
    transpose serialization) and scales W rows by bins/width_d so the
    fp16 matmul accumulates proj' = proj * bins/width in PSUM fp32.
  - ScalarE stages each PSUM tile to SBUF fp16 with a per-partition bias
    (Identity activation): u = proj' - min*bins/width in [0, 20].  This
    frees the PSUM slot after ~2us so the PE never stalls on compare
    progress, and makes all 19 bin edges integer constants.
  - cdf_b = #(u >= b) via fused compare+accumulate.  The DVE accumulate
    path runs at 1x regardless of dtype (HW-measured), so VectorE uses a
    hand-built triple-edge DVE op (registered below): the accumulator
    carries c(b) + 4096*c(b+5) (two exact counts packed into one fp32),
    while a third compare against b+10 (computed by the seed uop into a
    swap flop) keeps a running count in a spare pipeline stage and
    streams it to the out tensor -- its last element is the third count,
    harvested for all five passes with one strided copy per tile.
    15 edges in 5 VectorE passes; ScalarE handles edges 16..19 via Sign
    activation (+accum) plus the PSUM->SBUF staging.
  - Bin counts are cdf differences; bin 0 uses the constant shard total.
  - Histogram pieces AllReduce across the 8 cores as soon as their tiles
    finish (overlapping compute); the last piece is small so the
    trailing collective is short.  L2-normalize on device.
"""

import sys

if "/opt/trn_rl_repo" not in sys.path:
    sys.path.insert(0, "/opt/trn_rl_repo")

import numpy as np

S, IN_DIM = 16384, 1024
NUM_PROJ, PROJ_DIM, BINS = 100, 64, 20
D = NUM_PROJ * PROJ_DIM          # 6400
N_CORES = 8
S_SHARD = S // N_CORES           # 2048
NE = BINS - 1                    # 19 interior edges (b = 1..19)
NP3 = 5                          # triple-edge passes on VectorE
NA = 4                           # Sign edges 16..19 on ScalarE
PK = 4096.0                      # dual-edge packing factor

_CACHE = {}


def _cdf3_ref(in0, in1, s0, s1, imm2):
    x = in0.astype(np.float32)
    e3 = 2.0 * s1 - s0
    dual = (x >= s0).astype(np.float32) + (x >= s1).astype(np.float32) * imm2
    pre = np.cumsum((x >= e3).astype(np.float32).reshape(x.shape[0], -1),
                    axis=-1).reshape(x.shape)
    return pre, dual.reshape(x.shape[0], -1).sum(axis=-1, keepdims=True)


def _register_cdf3():
    """Triple-edge compare op: accum = c(s0) + 4096*c(s1) (packed dual),
    out stream = running count of (x >= 2*s1-s0) -- its last element is the
    third edge's count.  Built by extending the lowered dual program's free
    BYPASS stages (idempotent registration)."""
    import copy as _copy
    from operator import add

    from concourse import dve_ops
    from concourse.dve_spec import C0, C1, C2, Spec, Src0, Zero, lower
    from concourse.dve_uop import (AluInp, AluOp, DelayInp, DveOpSpec,
                                   InpSel, OutSel)

    if "CDF3_ANT" in dve_ops._SUB_OPCODE_FOR_NAME:
        return next(op for op in dve_ops.OPS if op.name == "CDF3_ANT")
    spec = Spec(
        body=(Src0 >= C0) + (Src0 >= C1) * C2,
        accum=add,
        accum_init=Zero,
        reference=_cdf3_ref,
    )
    seed, steady = (_copy.deepcopy(u) for u in lower(spec, ver="v3"))

    PA, CA = AluInp.PREV_ALU_OUT, AluInp.CURR_ALU_OUT
    PD = [AluInp.PREV_DELAY_0, AluInp.PREV_DELAY_1, AluInp.PREV_DELAY_2,
          AluInp.PREV_DELAY_3, AluInp.PREV_DELAY_4]

    # seed: compute e3 = C1+C1-C0, carry it on delay lane 0 (s2 captures
    # s1's ALU out there already), latch into s5's swap flop; keep s4's
    # zero-seeding of the dual accumulator; zero s6's scan flop via lane 4.
    seed.datapath_config[0].op = AluOp.ADD
    seed.datapath_config[0].alu_src0 = PD[2]      # C1
    seed.datapath_config[0].alu_src1 = PD[2]
    seed.datapath_config[1].op = AluOp.SUBTRACT
    seed.datapath_config[1].alu_src0 = PA         # 2*C1
    seed.datapath_config[1].alu_src1 = PD[1]      # C0
    seed.datapath_config[4].delay[0] = DelayInp.PREV_DELAY   # keep e3 on L0
    seed.datapath_config[5].op = AluOp.BYPASS
    seed.datapath_config[5].alu_src0 = PD[0]      # e3
    seed.datapath_config[5].alu_src1 = PD[0]
    seed.datapath_config[5].swap_enable = 1       # latch e3
    seed.datapath_config[6].op = AluOp.BYPASS
    seed.datapath_config[6].alu_src0 = PD[4]      # lane4 = zero at seed
    seed.datapath_config[6].alu_src1 = PD[4]
    # s5 outputs e3 (don't write A there); s6/s7 output zero and keep the
    # original A-reset-through-the-tail behaviour
    seed.datapath_config[5].alu_out_a_enable = 0

    # steady: lane 5 carries x (PD4); s5 compares vs the latched e3,
    # s6 keeps the running count (scan), s7 forwards it to the output.
    steady.inp[5] = InpSel.SRC_0
    steady.datapath_config[5].op = AluOp.IS_GE
    steady.datapath_config[5].alu_src0 = PD[4]    # x
    steady.datapath_config[5].alu_src1 = AluInp.CURR_SWAP_OUT
    steady.datapath_config[5].alu_out_a_enable = 0
    # the dual running sum (s4's ALU out) rides delay lane 0 to s7 so the
    # accumulator readback still sees it carried through the pipeline tail
    steady.datapath_config[5].delay[0] = DelayInp.PREV_ALU_OUT
    steady.datapath_config[6].op = AluOp.ADD
    steady.datapath_config[6].alu_src0 = CA       # running 3rd-edge count
    steady.datapath_config[6].alu_src1 = PA
    steady.datapath_config[6].alu_out_a_enable = 0
    steady.datapath_config[6].delay[0] = DelayInp.PREV_DELAY
    steady.datapath_config[7].op = AluOp.BYPASS
    steady.datapath_config[7].alu_src0 = PD[0]    # dual running sum
    steady.datapath_config[7].alu_src1 = PD[0]
    steady.datapath_config[7].alu_out_a_enable = 1
    steady.datapath_config[7].delay[1] = DelayInp.PREV_ALU_OUT  # scan value
    steady.out_enable = {k: (1 if k.name == "WR0_LO" else 0)
                         for k in steady.out_enable}
    steady.out = {k: OutSel.DELAY_1 for k in steady.out}

    for u in (seed, steady):
        u.validate("v3")
    row = max(dve_ops._SUB_OPCODE_FOR_NAME.values()) + 1
    assert row < 0x20
    compiled = DveOpSpec(name="CDF3_ANT", opcode=row, uops=[seed, steady],
                         rd1_en=False)

    class _TripleDveOp(dve_ops.DveOp):
        def compile(self, ver):
            assert ver == "v3", f"hand-built uops are v3-only, got {ver}"
            return compiled

    op = _TripleDveOp("CDF3_ANT", spec, subdim=False,
                      uops_sha={"v3": compiled.sha("v3")})
    dve_ops.OPS.append(op)
    dve_ops.CUSTOM_DVE_SPECS[op.name] = op.spec
    dve_ops._SUB_OPCODE_FOR_NAME[op.name] = row
    return op


def build(s_shard=S_SHARD, d=D, in_dim=IN_DIM, n_cores=N_CORES, debug=False):
    import concourse.bacc as bacc
    import concourse.bass as bass
    from concourse import mybir
    from concourse.tile import TileContext

    cdf3 = _register_cdf3()

    f32 = mybir.dt.float32
    f16 = mybir.dt.float16
    i32 = mybir.dt.int32
    nt = d // 128                # 50
    kc_n = in_dim // 128         # 8
    chw = min(512, s_shard)      # matmul moving-operand width
    nch_n = s_shard // chw       # 4

    nc = bacc.Bacc("TRN2", target_bir_lowering=False, debug=False,
                   num_devices=n_cores)

    xT_d = nc.dram_tensor("xT16", [kc_n, 128, s_shard], f16,
                          kind="ExternalInput")
    wT_d = nc.dram_tensor("wT16", [kc_n, 128, d], f16, kind="ExternalInput")
    bias_d = nc.dram_tensor("bias", [128, nt], f32, kind="ExternalInput")
    out_d = nc.dram_tensor("out", [d, BINS], f32, kind="ExternalOutput")
    # pieces of the histogram all-reduce independently, overlapping compute;
    # the last piece is small so the trailing collective is short
    pieces = [(0, 22), (22, 40), (40, 48), (48, nt)]
    cc_ins, cc_outs = [], []
    for i, (t0, t1) in enumerate(pieces):
        cc_ins.append(nc.dram_tensor(f"cc_in{i}", [128, (t1 - t0) * BINS], f32))
        cc_outs.append(nc.dram_tensor(f"cc_out{i}", [128, (t1 - t0) * BINS],
                                      f32, addr_space="Shared"))
    if debug:
        dbg_hist = nc.dram_tensor("dbg_hist", [128, nt, BINS], f32,
                                  kind="ExternalOutput")
        dbg_cdf = nc.dram_tensor("dbg_cdf", [128, nt, BINS + 1], f32,
                                 kind="ExternalOutput")

    with TileContext(nc) as tc:
        with (
            tc.tile_pool(name="singles", bufs=1) as singles,
            tc.tile_pool(name="sp_pool", bufs=4) as sp_pool,
            tc.tile_pool(name="ps_p", bufs=2, space="PSUM") as ps_p,
        ):
            bias_sb = singles.tile([128, nt], f32)
            nc.sync.dma_start(out=bias_sb, in_=bias_d[:, :])

            # ScalarE Sign biases (-b for b = 16..19), one column each
            negb = singles.tile([128, NA], f32)
            for j in range(NA):
                nc.vector.memset(negb[:, j:j + 1], float(-(16 + j)))

            # per-pass scan outputs (counts <= 2048 are fp16-exact);
            # one strided copy harvests all five last-elements per tile
            vout = singles.tile([128, NP3, s_shard], f16)
            trash_a = singles.tile([128, s_shard], f16)

            # per-engine cdf accumulators (separate tiles so Tile never
            # serializes VectorE against ScalarE on writes)
            accp = singles.tile([128, nt, NP3], f32)   # packed dual counts
            acc3 = singles.tile([128, nt, NP3], f32)   # harvested 3rd counts
            acc_a = singles.tile([128, nt, NA], f32)

            # ---- Phase 0: plain DMA of pre-transposed x shard and W ----
            xT = singles.tile([128, kc_n, s_shard], f16)
            wT = singles.tile([128, kc_n, d], f16)
            # 16 fat x transfers (2 KB per partition line) spread the load
            # across all DMA queues; tile 0 only needs x + the first W chunk
            for kc in range(kc_n):
                for sh in range(2):
                    sl = slice(sh * (s_shard // 2), (sh + 1) * (s_shard // 2))
                    nc.sync.dma_start(out=xT[:, kc, sl], in_=xT_d[kc, :, sl])
            # W streams lazily in per-tile chunks behind the x load: tile 0
            # is gated only on x (4.2 MB) + the first 128 W columns, and
            # later chunks always arrive well ahead of their tile.
            for d0 in range(0, d, 128):
                for kc in range(kc_n):
                    nc.sync.dma_start(out=wT[:, kc, d0:d0 + 128],
                                      in_=wT_d[kc, :, d0:d0 + 128])

            # normalization / unpack scratch
            cdfx = singles.tile([128, nt, BINS + 1], f32)
            nc.vector.memset(cdfx[:, :, 0:1], float(s_shard))
            nc.vector.memset(cdfx[:, :, BINS:BINS + 1], 0.0)
            ip_t = singles.tile([128, nt, NP3], i32)
            ip2_t = singles.tile([128, nt, NP3], i32)
            hi_t = singles.tile([128, nt, NP3], f32)
            hist = singles.tile([128, nt, BINS], f32)
            hsum = singles.tile([128, nt, BINS], f32)
            sq = singles.tile([128, nt, BINS], f32)
            n2 = singles.tile([128, nt], f32)
            y_t = singles.tile([128, nt], f32)
            iy = singles.tile([128, nt], f32)
            a_t = singles.tile([128, nt], f32)
            b_t = singles.tile([128, nt], f32)
            r_t = singles.tile([128, nt], f32)
            outn = singles.tile([128, nt, BINS], f32)
            out_v = out_d[:, :].rearrange("(t p) b -> p t b", p=128)

            def emit_cc(hi):
                """Unpack cdf partials for tau in [t0, t1) and kick off the
                cross-core all-reduce (runs on DMA/CC queues in background)."""
                t0, t1 = pieces[hi]
                sl = slice(t0, t1)
                # unpack: lo = accp & 0xFFF (exact: accp is integer-valued
                # fp32 < 2^23, i32 convert exact), mid = (accp - lo) / 4096;
                # third-edge counts arrive pre-unpacked via the scan harvest
                nc.vector.tensor_copy(ip_t[:, sl], accp[:, sl])
                nc.vector.tensor_scalar(ip2_t[:, sl], ip_t[:, sl], 0xFFF,
                                        None, op0=mybir.AluOpType.bitwise_and)
                nc.vector.tensor_copy(cdfx[:, sl, 1:1 + NP3], ip2_t[:, sl])
                nc.vector.tensor_tensor(out=hi_t[:, sl], in0=accp[:, sl],
                                        in1=cdfx[:, sl, 1:1 + NP3],
                                        op=mybir.AluOpType.subtract)
                nc.vector.tensor_scalar(cdfx[:, sl, 1 + NP3:1 + 2 * NP3],
                                        hi_t[:, sl], 1.0 / PK, None,
                                        op0=mybir.AluOpType.mult)
                nc.vector.tensor_copy(cdfx[:, sl, 1 + 2 * NP3:1 + 3 * NP3],
                                      acc3[:, sl])
                # ScalarE counts are sums of sign in {-1,0,1}:
                # cdf = 0.5*sgn + N/2
                nc.vector.tensor_scalar(
                    cdfx[:, sl, 1 + 3 * NP3:BINS], acc_a[:, sl],
                    0.5, float(s_shard) / 2,
                    op0=mybir.AluOpType.mult, op1=mybir.AluOpType.add)
                nc.vector.tensor_tensor(
                    out=hist[:, sl], in0=cdfx[:, sl, 0:BINS],
                    in1=cdfx[:, sl, 1:BINS + 1],
                    op=mybir.AluOpType.subtract)
                if debug and hi == len(pieces) - 1:
                    nc.sync.dma_start(out=dbg_hist[:, :, :], in_=hist)
                    nc.sync.dma_start(out=dbg_cdf[:, :, :], in_=cdfx)
                nc.sync.dma_start(
                    out=cc_ins[hi][:, :],
                    in_=hist[:, sl].rearrange("p a b -> p (a b)"))
                nc.gpsimd.collective_compute(
                    "AllReduce",
                    mybir.AluOpType.add,
                    replica_groups=[list(range(n_cores))],
                    ins=[cc_ins[hi][:, :]],
                    outs=[cc_outs[hi][:, :]],
                )
                nc.sync.dma_start(
                    out=hsum[:, sl].rearrange("p a b -> p (a b)"),
                    in_=cc_outs[hi][:, :])

            hsum_g = singles.tile([128, nt, BINS], f32)

            def emit_norm(t0, t1, anchor=None):
                """L2-normalize the summed histogram for tau in [t0, t1) and
                write the output slice. When `anchor` is a tile index, route
                hsum through a no-op add of that tile's accumulator data so
                the scheduler's cost model places the chain after that tile's
                compares -- the collective is then long finished and no
                engine FIFO stalls on it."""
                sl = slice(t0, t1)
                w = t1 - t0
                if anchor is not None:
                    g_ap = acc_a[:, anchor, NA - 1:NA]
                    g_b = bass.AP(tensor=g_ap.tensor, offset=g_ap.offset,
                                  ap=[g_ap.ap[0], [0, w], [0, BINS]])
                    nc.vector.scalar_tensor_tensor(
                        out=hsum_g[:, sl], in0=g_b, scalar=0.0,
                        in1=hsum[:, sl],
                        op0=mybir.AluOpType.mult, op1=mybir.AluOpType.add)
                    h_in = hsum_g
                else:
                    h_in = hsum
                nc.vector.tensor_tensor(out=sq[:, sl], in0=h_in[:, sl],
                                        in1=h_in[:, sl],
                                        op=mybir.AluOpType.mult)
                nc.vector.tensor_reduce(out=n2[:, sl], in_=sq[:, sl],
                                        axis=mybir.AxisListType.X,
                                        op=mybir.AluOpType.add)
                nc.scalar.sqrt(y_t[:, sl], n2[:, sl])
                nc.vector.reciprocal(iy[:, sl], y_t[:, sl])
                # one Newton step for rsqrt: r = iy * (1.5 - 0.5*n2*iy^2)
                nc.vector.tensor_tensor(out=a_t[:, sl], in0=iy[:, sl],
                                        in1=iy[:, sl],
                                        op=mybir.AluOpType.mult)
                nc.vector.tensor_tensor(out=b_t[:, sl], in0=a_t[:, sl],
                                        in1=n2[:, sl],
                                        op=mybir.AluOpType.mult)
                nc.vector.tensor_scalar(b_t[:, sl], b_t[:, sl], -0.5, 1.5,
                                        op0=mybir.AluOpType.mult,
                                        op1=mybir.AluOpType.add)
                nc.vector.tensor_tensor(out=r_t[:, sl], in0=iy[:, sl],
                                        in1=b_t[:, sl],
                                        op=mybir.AluOpType.mult)
                r_ap = r_t[:, sl]
                r_b = bass.AP(tensor=r_ap.tensor, offset=r_ap.offset,
                              ap=[r_ap.ap[0], r_ap.ap[1], [0, BINS]])
                nc.vector.tensor_tensor(out=outn[:, sl], in0=h_in[:, sl],
                                        in1=r_b, op=mybir.AluOpType.mult)
                nc.sync.dma_start(out=out_v[:, sl], in_=outn[:, sl])

            # ---- Phase 1: d-tiles ----
            for tau in range(nt):
                pp = ps_p.tile([128, s_shard], f32)
                for nch in range(nch_n):
                    for kc in range(kc_n):
                        nc.tensor.matmul(
                            pp[:, nch * chw:(nch + 1) * chw],
                            lhsT=wT[:, kc, tau * 128:(tau + 1) * 128],
                            rhs=xT[:, kc, nch * chw:(nch + 1) * chw],
                            start=(kc == 0),
                            stop=(kc == kc_n - 1),
                        )
                # Stage PSUM -> SBUF fp16 once, adding the per-dim bias:
                # u = proj' - min * bins/width  in [0, bins].  Frees the
                # PSUM slot after ~2us so the PE is never gated on compares.
                sp = sp_pool.tile([128, s_shard], f16)
                nc.scalar.activation(
                    out=sp, in_=pp,
                    func=mybir.ActivationFunctionType.Identity,
                    bias=bias_sb[:, tau:tau + 1], scale=1.0)
                # VectorE: triple passes (b, b+5, b+10): packed dual in
                # the accumulator, third count harvested from the scan
                # output's last element before the next pass reuses trash_v
                for j in range(NP3):
                    nc.vector._custom_dve(
                        cdf3,
                        out=vout[:, j, :],
                        in0=sp,
                        s0=float(1 + j),
                        s1=float(6 + j),
                        imm2=PK,
                        accum_out=accp[:, tau, j:j + 1],
                    )
                vlast = vout[:, 0:NP3, s_shard - 1:s_shard]
                vl = bass.AP(tensor=vlast.tensor, offset=vlast.offset,
                             ap=[vlast.ap[0], [s_shard, NP3]])
                a3 = acc3[:, tau:tau + 1, 0:NP3]
                a3f = bass.AP(tensor=a3.tensor, offset=a3.offset,
                              ap=[a3.ap[0], [1, NP3]])
                nc.vector.tensor_copy(a3f, vl)
                # ScalarE: edges b = 16..19 via Sign(u - b)
                for j in range(NA):
                    nc.scalar.activation(
                        out=trash_a,
                        in_=sp,
                        func=mybir.ActivationFunctionType.Sign,
                        bias=negb[:, j:j + 1],
                        scale=1.0,
                        accum_out=acc_a[:, tau, j:j + 1],
                    )
                for hi, (t0, t1) in enumerate(pieces):
                    if tau == t1 - 1:
                        emit_cc(hi)
            # normalization staged so the bulk runs during the last tiles'
            # compute (its collectives have long completed); only the last
            # piece's collective is actually waited on at the end
            emit_norm(0, 40, anchor=47)
            emit_norm(40, pieces[-2][1], anchor=nt - 1)
            emit_norm(pieces[-1][0], pieces[-1][1])

    nc.compile()
    return nc


def host_prep(x, W, mins, maxs, s_shard=S_SHARD, n_cores=N_CORES):
    d = W.shape[0]
    nt = d // 128
    kc_n = IN_DIM // 128
    mins64 = np.asarray(mins, dtype=np.float64)
    maxs64 = np.asarray(maxs, dtype=np.float64)
    scale = float(BINS) / (maxs64 - mins64)                        # [d]
    # W rows scaled so the matmul yields proj' = proj * bins/width
    Ws = np.asarray(W, dtype=np.float64) * scale[:, None]
    w16T = np.ascontiguousarray(Ws.T.astype(np.float16))           # [in, d]
    w16T = w16T.reshape(kc_n, 128, d)
    x16T = np.ascontiguousarray(np.asarray(x, dtype=np.float16).T)  # [in, S]
    x16T = x16T.reshape(kc_n, 128, S)
    bias = (-mins64 * scale).astype(np.float32)                    # [d]
    bias_l = np.ascontiguousarray(bias.reshape(nt, 128).T)         # [128, nt]
    in_maps = []
    for i in range(n_cores):
        in_maps.append({
            "xT16": np.ascontiguousarray(
                x16T[:, :, i * s_shard:(i + 1) * s_shard]),
            "wT16": w16T,
            "bias": bias_l,
        })
    return in_maps


def run(x, W, mins, maxs, trace=False, **trace_kw):
    """Returns (output [100, 64, 20] f32, BassKernelResults)."""
    from concourse.bass_utils import run_bass_kernel_spmd

    if "nc" not in _CACHE:
        _CACHE["nc"] = build()
    nc = _CACHE["nc"]
    in_maps = host_prep(x, W, mins, maxs)
    res = run_bass_kernel_spmd(nc, in_maps, core_ids=list(range(N_CORES)),
                               trace=trace, **trace_kw)
    out = res.results[0]["out"].reshape(NUM_PROJ, PROJ_DIM, BINS)
    return np.asarray(out, dtype=np.float32), res


def kernel(x, W, mins, maxs, num_of_projection=NUM_PROJ, bins=BINS):
    assert int(num_of_projection) == NUM_PROJ and int(bins) == BINS
    out, _ = run(x, W, mins, maxs, trace=False)
    return out
